# revision 1
# baseline (speedup 1.0000x reference)
"""Trainium2 Bass kernel for nn_AttentionHead (B=4, T=2048, D=1024, HS=64).

Strategy (8 NeuronCores, SPMD):
  - Sequence-shard the query/t axis: core m owns t in [m*256, (m+1)*256).
    Each core holds a [256, 2048, 64] slice of rel_pos_embd (the 1 GiB
    stream is read exactly once chip-wide -> memory-roofline sharding).
  - Host prep: rel_pos slice pre-transposed to [t-pair, 2*HS, T] so the
    contraction dim (c) lands on SBUF partitions with no on-chip
    transposes; x pre-transposed to [B, D, T]; sqrt(HS) folded into wk/bk.
  - Per core: q/k projections + QK^T in fp32 (softmax logits have std ~26,
    i.e. softmax ~= argmax; fp22 rounding there can flip argmax rows). The
    rel-bias einsum q . rel_pos runs in float32r (full-rate FP22) where
    operand magnitudes keep the logit error ~1e-5.
  - Mixed-partition PSUM layout p = 32*(jg%4) + 8*(jg//4) + 4*dl + b lets
    the per-t rel matmuls (K-packed: two consecutive t's stacked on the
    contraction axis; zero-padded lhsT columns shift output rows within a
    32-strip so the PSUM base partition stays 32-aligned) accumulate into
    the same PSUM tile as b-pair-packed QK^T matmuls.
  - Softmax along the free axis; attention transposed 128x128 via PE;
    AV matmuls use strided lhsT views to de-interleave b.
"""

import numpy as np

import concourse.bass as bass
import concourse.mybir as mybir
import concourse.tile as tile
from concourse import bacc

F32 = mybir.dt.float32
F32R = mybir.dt.float32r
AX = mybir.AxisListType.X
EXP = mybir.ActivationFunctionType.Exp

B, T, D, HS = 4, 2048, 1024, 64
NCORES = 8
TL = T // NCORES          # 256 query rows per core
G = TL // 32              # 8 groups of 32 t's
NP = 128
DC = D // NP              # 8 contraction chunks
NVB = T // 512            # 4 psum banks per score row-block
NCI = T // NP             # 16 v-chunks for AV

RP_BUFS = 6               # rel_pos stream prefetch depth (1 MB tiles)


def build_program(dbg=False):
    nc = bacc.Bacc(None, target_bir_lowering=False, debug=True)

    rp_d = nc.dram_tensor("relposT2", [TL // 2, 2 * HS, T], F32R, kind="ExternalInput")
    xT_d = nc.dram_tensor("xT", [B, D, T], F32, kind="ExternalInput")
    xq_d = nc.dram_tensor("xq", [D, TL * B], F32, kind="ExternalInput")
    wq_d = nc.dram_tensor("wq", [D, HS], F32, kind="ExternalInput")
    wk_d = nc.dram_tensor("wk8", [D, HS], F32, kind="ExternalInput")
    wv_d = nc.dram_tensor("wv", [D, HS], F32, kind="ExternalInput")
    bq_d = nc.dram_tensor("bq2", [NP, 1], F32, kind="ExternalInput")
    bk_d = nc.dram_tensor("bk8_2", [NP, 1], F32, kind="ExternalInput")
    bv_d = nc.dram_tensor("bv_rep", [NP, HS], F32, kind="ExternalInput")
    id_d = nc.dram_tensor("identity", [NP, NP], F32, kind="ExternalInput")
    rqz_d = nc.dram_tensor("relq_zero", [NP, 2048], F32R, kind="ExternalInput")
    out_d = nc.dram_tensor("out_raw", [NP, G * HS], F32, kind="ExternalOutput")
    if dbg:
        dbg_lhsTs = nc.dram_tensor("dbg_lhsTs", [NP, G * 256], F32, kind="ExternalOutput")
        dbg_kT2 = nc.dram_tensor("dbg_kT2", [NP, 2 * T], F32, kind="ExternalOutput")
        dbg_Vbuf = nc.dram_tensor("dbg_Vbuf", [NP, B * NCI * HS], F32, kind="ExternalOutput")
        dbg_att0 = nc.dram_tensor("dbg_att0", [NP, T], F32, kind="ExternalOutput")

    with tile.TileContext(nc) as tc:
        with tc.tile_pool(name="const", bufs=1) as const, \
             tc.tile_pool(name="persist", bufs=1) as persist, \
             tc.tile_pool(name="xtp", bufs=3) as xtp, \
             tc.tile_pool(name="rpp", bufs=RP_BUFS) as rpp, \
             tc.tile_pool(name="attp", bufs=2) as attp, \
             tc.tile_pool(name="attTp", bufs=4) as attTp, \
             tc.tile_pool(name="smx", bufs=2) as smx:

            # ---- constants ----
            wq_sb = const.tile([NP, DC * HS], F32, tag="wq", name="wq")
            wk_sb = const.tile([NP, DC * HS], F32, tag="wk", name="wk")
            wv_sb = const.tile([NP, DC * HS], F32, tag="wv", name="wv")
            for w_sb, w_d in ((wq_sb, wq_d), (wk_sb, wk_d), (wv_sb, wv_d)):
                nc.sync.dma_start(
                    out=w_sb[:].rearrange("p (dc h) -> p dc h", dc=DC),
                    in_=w_d[:, :].rearrange("(dc p) h -> p dc h", p=NP),
                )
            bq_sb = const.tile([NP, 1], F32, tag="bq", name="bq")
            bk_sb = const.tile([NP, 1], F32, tag="bk", name="bk")
            bv_sb = const.tile([NP, HS], F32, tag="bv", name="bv")
            id_sb = const.tile([NP, NP], F32, tag="iden", name="iden")
            nc.sync.dma_start(out=bq_sb[:], in_=bq_d[:, :])
            nc.sync.dma_start(out=bk_sb[:], in_=bk_d[:, :])
            nc.sync.dma_start(out=bv_sb[:], in_=bv_d[:, :])
            nc.sync.dma_start(out=id_sb[:], in_=id_d[:, :])

            # ---- persistent activations ----
            qTp = persist.tile([NP, TL * B], F32, tag="qTp", name="qTp")          # [128, 1024]
            relqS = persist.tile([NP, 2048], F32R, tag="relqS", name="relqS")      # grid lhsT buffer
            lhsTs = persist.tile([NP, G * 256], F32, tag="lhsTs", name="lhsTs")     # [128, 2048]
            kT2 = persist.tile([NP, 2 * T], F32, tag="kT2", name="kT2")           # [128, 4096]
            Vbuf = persist.tile([NP, B * NCI * HS], F32, tag="Vbuf", name="Vbuf")  # [128, 4096]
            outbuf = persist.tile([NP, G * HS], F32, tag="outbuf", name="outbuf")    # [128, 512]

            nc.sync.dma_start(out=relqS[:], in_=rqz_d[:, :])
            nc.gpsimd.memset(lhsTs[:], 0.0)

            # =========== stage A: q projection ===========
            # psq blocks hold qT cols duplicated in both partition halves:
            # psq[blk][64*dl + h, p_local], qT col = 512*blk + p_local,
            # p_local = 128*g4 + 32*ji + 8*jj + 4*dl' + b
            with tc.tile_pool(name="xqp", bufs=2) as xqp, \
                 tc.tile_pool(name="qps", bufs=2, space="PSUM") as qps:
                psq = [qps.tile([NP, 512], F32, tag=f"psq{blk}", name=f"psq{blk}") for blk in range(2)]
                for dc in range(DC):
                    xq_t = xqp.tile([NP, TL * B], F32, tag="xqt", name="xqt")
                    nc.sync.dma_start(out=xq_t[:], in_=xq_d[dc * NP:(dc + 1) * NP, :])
                    for blk in range(2):
                        for half in range(2):
                            nc.tensor.matmul(
                                psq[blk][64 * half:64 * half + 64, :],
                                lhsT=wq_sb[:, HS * dc:HS * (dc + 1)],
                                rhs=xq_t[:, 512 * blk:512 * (blk + 1)],
                                start=(dc == 0), stop=(dc == DC - 1),
                                skip_group_check=True,
                            )
                # epilogue 1: qTp = psq + bq (plain q, both partition halves)
                for blk in range(2):
                    for dl in range(2):
                        rows = slice(64 * dl, 64 * dl + 64)
                        nc.vector.tensor_scalar_add(
                            qTp[rows, 512 * blk:512 * (blk + 1)],
                            psq[blk][rows, :], bq_sb[rows, 0:1])
                # epilogue 2: scatter q into lhsTs (b-pair-packed scores lhsT)
                # lhsTs col = 256*g + 128*pair + 4*c + b_sel, row half dl <-> b_sel
                for blk in range(2):
                    src4 = psq[blk][:].rearrange("p (g c b) -> p g c b", g=4, c=32)
                    dst5 = lhsTs[:].rearrange("p (g pr c b) -> p g pr c b", g=G, pr=2, c=32)
                    for pair in range(2):
                        for dl in range(2):
                            b_sel = 2 * pair + dl
                            rows = slice(64 * dl, 64 * dl + 64)
                            src = src4[rows, :, :, b_sel]                # [64,4,32]
                            dst = dst5[rows, 4 * blk:4 * blk + 4, pair, :, b_sel]
                            nc.vector.tensor_scalar_add(dst, src, bq_sb[rows, 0:1])

            # =========== stage B: k / v projections (full T, all b) ===========
            with tc.tile_pool(name="kps", bufs=1, space="PSUM") as kps, \
                 tc.tile_pool(name="vps", bufs=2, space="PSUM") as vps:
                for bp in range(2):
                    psk = kps.tile([NP, T], F32, tag="psk", name="psk")
                    psvs = []
                    for b in (2 * bp, 2 * bp + 1):
                        hb = b % 2
                        psv = vps.tile([NP, NCI * HS], F32, tag="psv", name="psv")
                        psvs.append(psv)
                        for dc in range(DC):
                            xt = xtp.tile([NP, T], F32, tag="xt", name="xt")
                            nc.sync.dma_start(
                                out=xt[:], in_=xT_d[b, dc * NP:(dc + 1) * NP, :])
                            for vb in range(NVB):
                                nc.tensor.matmul(
                                    psk[64 * hb:64 * hb + 64, 512 * vb:512 * (vb + 1)],
                                    lhsT=wk_sb[:, HS * dc:HS * (dc + 1)],
                                    rhs=xt[:, 512 * vb:512 * (vb + 1)],
                                    start=(dc == 0), stop=(dc == DC - 1),
                                    skip_group_check=True,
                                )
                            for ci in range(NCI):
                                # one start=True per (bank, partition-set): it
                                # poisons the whole bank as pending-zero, so
                                # later ci's first write overwrites correctly
                                nc.tensor.matmul(
                                    psv[:, HS * ci:HS * (ci + 1)],
                                    lhsT=xt[:, NP * ci:NP * (ci + 1)],
                                    rhs=wv_sb[:, HS * dc:HS * (dc + 1)],
                                    start=(dc == 0 and ci % 8 == 0),
                                    stop=(dc == DC - 1),
                                    skip_group_check=True,
                                )
                        # V epilogue (psv banks are private to this b)
                        for half in range(2):
                            nc.vector.tensor_copy(
                                out=Vbuf[:, 1024 * b + 512 * half:1024 * b + 512 * (half + 1)],
                                in_=psv[:, 512 * half:512 * (half + 1)])
                    # k epilogue for both b of the pair (after both halves filled)
                    for b in (2 * bp, 2 * bp + 1):
                        hb = b % 2
                        rows = slice(64 * hb, 64 * hb + 64)
                        for vb in range(NVB):
                            nc.vector.tensor_scalar_add(
                                kT2[rows, T * bp + 512 * vb:T * bp + 512 * (vb + 1)],
                                psk[rows, 512 * vb:512 * (vb + 1)],
                                bk_sb[rows, 0:1])

            # =========== stage C: scores + rel + softmax + AV ===========
            with tc.tile_pool(name="sp", bufs=1, space="PSUM") as spool, \
                 tc.tile_pool(name="atps", bufs=2, space="PSUM") as atps, \
                 tc.tile_pool(name="avps", bufs=2, space="PSUM") as avps:

                deferred = []     # work items from the previous group

                def make_deferred(g, att, avp):
                    items = []
                    for ci in range(NCI):
                        def item(ci=ci, att=att, avp=avp):
                            pst = atps.tile([NP, NP], F32, tag="pst", name="pst")
                            nc.tensor.transpose(
                                pst[:], att[:, NP * ci:NP * (ci + 1)], id_sb[:])
                            attT = attTp.tile([NP, NP], F32, tag="attT", name="attT")
                            nc.vector.tensor_copy(out=attT[:], in_=pst[:])
                            attT4 = attT[:].rearrange("v (c b) -> v c b", b=4)
                            for b in range(4):
                                nc.tensor.matmul(
                                    avp[32 * b:32 * b + 32, :],
                                    lhsT=attT4[:, :, b],
                                    rhs=Vbuf[:, 1024 * b + HS * ci:1024 * b + HS * (ci + 1)],
                                    start=(ci == 0), stop=(ci == NCI - 1),
                                    skip_group_check=True,
                                    tile_position=(0, 32 * b),
                                )
                        items.append(item)

                    def epilogue(g=g, avp=avp):
                        nc.vector.tensor_add(
                            out=outbuf[:, HS * g:HS * (g + 1)], in0=avp[:], in1=bv_sb[:])
                    items.append(epilogue)
                    return items

                if dbg:
                    nc.sync.dma_start(out=dbg_lhsTs[:, :], in_=lhsTs[:])
                    nc.sync.dma_start(out=dbg_kT2[:, :], in_=kT2[:])
                    nc.sync.dma_start(out=dbg_Vbuf[:, :], in_=Vbuf[:])
                for g in range(G):
                    sp = spool.tile([NP, T], F32, tag="sp", name="sp")
                    # scores (fp32, b-pair packed)
                    for pair in range(2):
                        for vb in range(NVB):
                            nc.tensor.matmul(
                                sp[:, 512 * vb:512 * (vb + 1)],
                                lhsT=lhsTs[:, 256 * g + 128 * pair:256 * g + 128 * (pair + 1)],
                                rhs=kT2[:, T * pair + 512 * vb:T * pair + 512 * (vb + 1)],
                                start=(pair == 0), stop=False,
                                skip_group_check=True,
                            )
                    # refresh relqS data windows for this group
                    # window jg data at cols [128*jg, 128*jg+8), grid slot 16*jg
                    # dst col = 512*jj + 128*ji + 4*dl + b ; src = qTp group block
                    for dl in range(2):
                        rows = slice(64 * dl, 64 * dl + 64)
                        srcq = qTp[rows, NP * g:NP * (g + 1)].rearrange(
                            "p (ji jj dlb b) -> p ji jj dlb b", ji=4, jj=4, dlb=2)
                        srcq = srcq[:, :, :, dl, :].transpose([0, 2, 1, 3])  # [64,jj,ji,b]
                        dstq = relqS[rows, :].rearrange(
                            "p (jj ji r) -> p jj ji r", jj=4, ji=4)[:, :, :, 4 * dl:4 * dl + 4]
                        nc.vector.tensor_copy(out=dstq, in_=srcq)
                    # rel bias (float32r) + drain deferred work of group g-1
                    relq3 = relqS[:].rearrange("p (k r) -> p k r", r=8)
                    for jg in range(16):
                        j = 16 * g + jg
                        rpt = rpp.tile([NP, T], F32R, tag="rpt", name="rpt")
                        nc.sync.dma_start(out=rpt[:], in_=rp_d[j, :, :])
                        a_jg = 4 * (jg % 4) + jg // 4
                        k0 = 16 * jg - a_jg
                        for vb in range(NVB):
                            nc.tensor.matmul(
                                sp[:, 512 * vb:512 * (vb + 1)],
                                lhsT=relq3[:, k0:k0 + 16, :],
                                rhs=rpt[:, 512 * vb:512 * (vb + 1)],
                                start=False, stop=(jg == 15 and vb == NVB - 1),
                                skip_group_check=True,
                            )
                        if deferred:
                            deferred.pop(0)()
                    while deferred:
                        deferred.pop(0)()
                    # softmax over the free axis
                    mx4 = smx.tile([NP, NVB], F32, tag="mx4", name="mx4")
                    ngm = smx.tile([NP, 1], F32, tag="ngm", name="ngm")
                    z4 = smx.tile([NP, NVB], F32, tag="z4", name="z4")
                    zs = smx.tile([NP, 1], F32, tag="zs", name="zs")
                    rz = smx.tile([NP, 1], F32, tag="rz", name="rz")
                    for vb in range(NVB):
                        nc.vector.reduce_max(
                            out=mx4[:, vb:vb + 1], in_=sp[:, 512 * vb:512 * (vb + 1)], axis=AX)
                    nc.vector.reduce_max(out=ngm[:], in_=mx4[:], axis=AX, negate=True)
                    att = attp.tile([NP, T], F32, tag="att", name="att")
                    for vb in range(NVB):
                        nc.scalar.activation(
                            out=att[:, 512 * vb:512 * (vb + 1)],
                            in_=sp[:, 512 * vb:512 * (vb + 1)],
                            func=EXP, bias=ngm[:, 0:1], scale=1.0)
                    for vb in range(NVB):
                        nc.vector.reduce_sum(
                            out=z4[:, vb:vb + 1], in_=att[:, 512 * vb:512 * (vb + 1)], axis=AX)
                    nc.vector.reduce_sum(out=zs[:], in_=z4[:], axis=AX)
                    nc.vector.reciprocal(rz[:], zs[:])
                    nc.vector.tensor_scalar_mul(att[:], att[:], rz[:, 0:1])
                    # queue AV work; drained during group g+1's rel loop
                    if dbg and g == 0:
                        nc.sync.dma_start(out=dbg_att0[:, :], in_=att[:])
                    avp = avps.tile([NP, HS], F32, tag="avp", name="avp")
                    deferred = make_deferred(g, att, avp)
                    if g == G - 1:
                        while deferred:
                            deferred.pop(0)()

                nc.sync.dma_start(out=out_d[:, :], in_=outbuf[:])

    nc.finalize()
    return nc


# ---------------- host side ----------------

def host_prep(x, wq, bq, wk, bk, wv, bv, rel_pos_embd):
    """Build the 8 per-core input dicts."""
    x = np.ascontiguousarray(np.asarray(x, np.float32))
    rel = np.asarray(rel_pos_embd, np.float32)
    wq = np.ascontiguousarray(np.asarray(wq, np.float32))
    wv = np.ascontiguousarray(np.asarray(wv, np.float32))
    s = np.float32(np.sqrt(np.float32(HS)))
    wk8 = np.ascontiguousarray(np.asarray(wk, np.float32) * s)
    bk8 = np.asarray(bk, np.float32) * s
    bq = np.asarray(bq, np.float32)
    bv = np.asarray(bv, np.float32)

    xT = np.ascontiguousarray(x.transpose(0, 2, 1))          # [B, D, T]
    bq2 = np.ascontiguousarray(np.tile(bq[None, :], (2, 1)).reshape(NP, 1))
    bk2 = np.ascontiguousarray(np.tile(bk8[None, :], (2, 1)).reshape(NP, 1))
    bv_rep = np.ascontiguousarray(np.tile(bv[None, :], (NP, 1)))
    iden = np.eye(NP, dtype=np.float32)
    relq_zero = np.zeros((NP, 2048), np.float32)

    in_maps = []
    for m in range(NCORES):
        t0 = m * TL
        rp = rel[t0:t0 + TL]                                  # [TL, T, HS]
        relposT2 = np.ascontiguousarray(
            rp.transpose(0, 2, 1).reshape(TL // 2, 2 * HS, T))
        # xq col 128*g + p, p = 32*ji + 8*jj + 4*dl + b,
        # t = t0 + 32*g + 8*jj + 2*ji + dl
        xs = x[:, t0:t0 + TL, :].reshape(B, G, 4, 4, 2, D)    # [b,g,jj,ji,dl,d]
        xq = np.ascontiguousarray(
            xs.transpose(5, 1, 3, 2, 4, 0).reshape(D, G * 128))
        in_maps.append(dict(
            relposT2=relposT2, xT=xT, xq=xq,
            wq=wq, wk8=wk8, wv=wv,
            bq2=bq2, bk8_2=bk2, bv_rep=bv_rep, identity=iden,
            relq_zero=relq_zero,
        ))
    return in_maps


def host_unshard(raws):
    """raws: list of 8 out_raw [128, G*HS] -> full [B, T, HS]."""
    out = np.empty((B, T, HS), np.float32)
    for m in range(NCORES):
        t0 = m * TL
        # p2 = 32*b + 8*ji + 2*jj + dl ; col = 64*g + h
        # t = t0 + 32*g + 8*jj + 2*ji + dl
        r = np.asarray(raws[m], np.float32).reshape(4, 4, 4, 2, G, HS)
        out[:, t0:t0 + TL, :] = r.transpose(0, 4, 2, 1, 3, 5).reshape(B, TL, HS)
    return out


_NC_CACHE = []


def kernel(**inputs) -> np.ndarray:
    from concourse.bass_utils import run_bass_kernel_spmd

    if not _NC_CACHE:
        _NC_CACHE.append(build_program())
    nc = _NC_CACHE[0]
    in_maps = host_prep(**inputs)
    res = run_bass_kernel_spmd(nc, in_maps, core_ids=list(range(NCORES)))
    raws = [res.results[i]["out_raw"] for i in range(NCORES)]
    return host_unshard(raws)



# revision 3
# speedup vs baseline: 2.5610x; 2.5610x over previous
"""Trainium2 Bass kernel for nn_AttentionHead (B=4, T=2048, D=1024, HS=64).

v2: low-precision rebuild of the sequence-sharded baseline.

Strategy (8 NeuronCores, SPMD, core m owns query rows t in [m*256, (m+1)*256)):
  - rel_pos_embd slice streamed as fp8 e4m3 (x8 scale on host): 32 MiB/core
    instead of 128 MiB.  The rel bias einsum runs as DoubleRow fp8 matmuls
    (2 k-tiles per pass, 0.5 cyc/row): lhsT = zero-padded q/8 in fp8.
  - x streamed as fp16 [B, D, T] (16 MiB); q/k/v projections in fp16
    (fp16 products are exact in the PE's fp22 pipeline; only the fp16
    rounding of x and w itself perturbs scores, ~5e-3 logit noise).
  - scores QK^T in fp16 with the b-pair contraction-packing trick
    (128-deep contraction = 2 b's x 64 c, zero-padded lhsT halves).
  - softmax in fp32 PSUM -> fp16 att; att transposed via PE (fp16
    identity); AV in fp16 with strided attT views de-interleaving b.

PSUM row layout per 32-t group: p = 4*tt + b. Output raw[32*b+tt, 64*g+h].
"""

import numpy as np
import ml_dtypes

import concourse.bass as bass
import concourse.mybir as mybir
import concourse.tile as tile
from concourse import bacc
from concourse.ap import AP

F32 = mybir.dt.float32
F16 = mybir.dt.float16
F8 = mybir.dt.float8e4
AX = mybir.AxisListType.X
EXP = mybir.ActivationFunctionType.Exp
ADD = mybir.AluOpType.add
MULT = mybir.AluOpType.mult
DR = mybir.MatmulPerfMode.DoubleRow

B, T, D, HS = 4, 2048, 1024, 64
NCORES = 8
TL = T // NCORES          # 256 query rows per core
G = TL // 32              # 8 groups of 32 t's
NQ = TL // 4              # 64 rel t-quads per core (8 per group)
NP = 128
DC = D // NP              # 8 contraction chunks
NVB = 4                   # 512-col blocks in T
RP_BUFS = 14              # rel quad prefetch depth (512 KiB fp8 tiles)


def build_program(dbg=False):
    nc = bacc.Bacc(None, target_bir_lowering=False, debug=True)

    rp_d = nc.dram_tensor("relp8", [NQ, NP, 2 * T], F8, kind="ExternalInput")
    xT_d = nc.dram_tensor("xT16", [B, D, T], F16, kind="ExternalInput")
    wq_d = nc.dram_tensor("wq16", [D, HS], F16, kind="ExternalInput")
    wk_d = nc.dram_tensor("wk16", [D, HS], F16, kind="ExternalInput")
    wv_d = nc.dram_tensor("wv16", [D, HS], F16, kind="ExternalInput")
    bq_d = nc.dram_tensor("bq2", [NP, 1], F32, kind="ExternalInput")
    bk_d = nc.dram_tensor("bk8_2", [NP, 1], F32, kind="ExternalInput")
    bv_d = nc.dram_tensor("bv_rep", [NP, HS], F32, kind="ExternalInput")
    id_d = nc.dram_tensor("iden16", [NP, NP], F16, kind="ExternalInput")
    out_d = nc.dram_tensor("out_raw", [NP, G * HS], F32, kind="ExternalOutput")
    if dbg:
        dbg_L = nc.dram_tensor("dbg_L", [NP, G * 256], F32, kind="ExternalOutput")
        dbg_rq = nc.dram_tensor("dbg_rq", [NP, NQ * 128], F32, kind="ExternalOutput")
        dbg_kT2 = nc.dram_tensor("dbg_kT2", [NP, 2 * T], F32, kind="ExternalOutput")
        dbg_att0 = nc.dram_tensor("dbg_att0", [NP, T], F32, kind="ExternalOutput")

    # One program runs on all 8 cores; the per-core q t-slice comes from
    # per-core data (xq16 = xT16[:, :, t0:t0+TL]).
    xq_d = nc.dram_tensor("xq16", [B, D, TL], F16, kind="ExternalInput")

    with tile.TileContext(nc) as tc:
        with tc.tile_pool(name="const", bufs=1) as const, \
             tc.tile_pool(name="persist", bufs=1) as persist, \
             tc.tile_pool(name="xtp", bufs=3) as xtp, \
             tc.tile_pool(name="rpp", bufs=RP_BUFS) as rpp, \
             tc.tile_pool(name="attp", bufs=2) as attp, \
             tc.tile_pool(name="attTp", bufs=4) as attTp, \
             tc.tile_pool(name="smx", bufs=2) as smx:

            # ---- constants ----
            wq_sb = const.tile([NP, DC * HS], F16, tag="wq", name="wq")
            wk_sb = const.tile([NP, DC * HS], F16, tag="wk", name="wk")
            wv_sb = const.tile([NP, DC * HS], F16, tag="wv", name="wv")
            for w_sb, w_d in ((wq_sb, wq_d), (wk_sb, wk_d), (wv_sb, wv_d)):
                nc.sync.dma_start(
                    out=w_sb[:].rearrange("p (dc h) -> p dc h", dc=DC),
                    in_=w_d[:, :].rearrange("(dc p) h -> p dc h", p=NP),
                )
            bq_sb = const.tile([NP, 1], F32, tag="bq", name="bq")
            bk_sb = const.tile([NP, 1], F32, tag="bk", name="bk")
            bv_sb = const.tile([NP, HS], F32, tag="bv", name="bv")
            id_sb = const.tile([NP, NP], F16, tag="iden", name="iden")
            nc.sync.dma_start(out=bq_sb[:], in_=bq_d[:, :])
            nc.sync.dma_start(out=bk_sb[:], in_=bk_d[:, :])
            nc.sync.dma_start(out=bv_sb[:], in_=bv_d[:, :])
            nc.sync.dma_start(out=id_sb[:], in_=id_d[:, :])

            # ---- persistent activations ----
            kT2 = persist.tile([NP, 2 * T], F16, tag="kT2", name="kT2")
            Vbuf = persist.tile([NP, B * 16 * HS], F16, tag="Vbuf", name="Vbuf")
            Lsc = persist.tile([NP, G * 256], F16, tag="Lsc", name="Lsc")
            relq = persist.tile([NP, NQ * 256], F8, tag="relq", name="relq")
            outbuf = persist.tile([NP, G * HS], F32, tag="outbuf", name="outbuf")
            nc.gpsimd.memset(Lsc[:], 0.0)
            nc.gpsimd.memset(relq[:], 0.0)

            LSC_PITCH = G * 256
            RELQ_PITCH = NQ * 256

            # =========== stage AB: q/k/v projections ===========
            with tc.tile_pool(name="qps", bufs=2, space="PSUM") as qps, \
                 tc.tile_pool(name="kps", bufs=1, space="PSUM") as kps, \
                 tc.tile_pool(name="vps", bufs=1, space="PSUM") as vps:
                for bp in range(2):
                    psk = kps.tile([NP, T], F32, tag="psk", name="psk")
                    for b in (2 * bp, 2 * bp + 1):
                        h = b % 2
                        psq = qps.tile([NP, TL], F32, tag="psq", name="psq")
                        psv = vps.tile([NP, 16 * HS], F32, tag="psv", name="psv")
                        for dc in range(DC):
                            xt = xtp.tile([NP, T], F16, tag="xt", name="xt")
                            nc.sync.dma_start(
                                out=xt[:], in_=xT_d[b, dc * NP:(dc + 1) * NP, :])
                            xq = xtp.tile([NP, TL], F16, tag="xq", name="xq")
                            nc.sync.dma_start(
                                out=xq[:], in_=xq_d[b, dc * NP:(dc + 1) * NP, :])
                            for half in range(2):
                                nc.tensor.matmul(
                                    psq[64 * half:64 * half + 64, :],
                                    lhsT=wq_sb[:, HS * dc:HS * (dc + 1)],
                                    rhs=xq[:],
                                    start=(dc == 0), stop=(dc == DC - 1),
                                    skip_group_check=True,
                                    tile_position=(0, 64 * half),
                                )
                            for vb in range(NVB):
                                nc.tensor.matmul(
                                    psk[64 * h:64 * h + 64, 512 * vb:512 * (vb + 1)],
                                    lhsT=wk_sb[:, HS * dc:HS * (dc + 1)],
                                    rhs=xt[:, 512 * vb:512 * (vb + 1)],
                                    start=(dc == 0), stop=(dc == DC - 1),
                                    skip_group_check=True,
                                    tile_position=(0, 64 * h),
                                )
                            for ci in range(16):
                                nc.tensor.matmul(
                                    psv[:, HS * ci:HS * (ci + 1)],
                                    lhsT=xt[:, NP * ci:NP * (ci + 1)],
                                    rhs=wv_sb[:, HS * dc:HS * (dc + 1)],
                                    start=(dc == 0 and ci % 8 == 0),
                                    stop=(dc == DC - 1),
                                    skip_group_check=True,
                                )
                        # V epilogue: copy to fp16 (bias added at the end)
                        nc.vector.tensor_copy(
                            out=Vbuf[:, 1024 * b:1024 * (b + 1)], in_=psv[:])
                        # q epilogue: scatter into Lsc (fp16, +bq) and
                        # relq (fp8, (q+bq)/8)
                        for hh in range(2):
                            rows = slice(64 * hh, 64 * hh + 64)
                            if hh == h:
                                dst = AP(Lsc[:].tensor,
                                         64 * hh * LSC_PITCH + 128 * (b // 2) + b,
                                         [[LSC_PITCH, 64], [256, G], [4, 32]])
                                src = AP(psq[:].tensor, 64 * hh * TL,
                                        [[TL, 64], [32, G], [1, 32]])
                                nc.vector.tensor_scalar_add(
                                    dst, src, bq_sb[rows, 0:1])
                            # relq col = 256*j + 128*i + m,
                            # m = 16*jq + 8*i + 4*tp + b, j = 8*g + jq
                            tp = hh
                            dstq = AP(relq[:].tensor,
                                      64 * tp * RELQ_PITCH + 4 * tp + b,
                                      [[RELQ_PITCH, 64], [2048, G],
                                       [272, 8], [136, 2]])
                            srcq = AP(psq[:].tensor, 64 * tp * TL + tp,
                                      [[TL, 64], [32, G], [4, 8], [2, 2]])
                            nc.vector.tensor_scalar(
                                dstq, srcq, bq_sb[rows, 0:1], 0.125,
                                op0=ADD, op1=MULT)
                    # K epilogue for the pair
                    for b in (2 * bp, 2 * bp + 1):
                        h = b % 2
                        rows = slice(64 * h, 64 * h + 64)
                        nc.vector.tensor_scalar_add(
                            kT2[rows, T * bp:T * (bp + 1)],
                            psk[rows, :], bk_sb[rows, 0:1])

            if dbg:
                nc.sync.dma_start(out=dbg_L[:, :], in_=Lsc[:])
                nc.sync.dma_start(out=dbg_rq[:, :], in_=relq[:])
                nc.sync.dma_start(out=dbg_kT2[:, :], in_=kT2[:])

            # =========== stage C: scores + rel + softmax + AV ===========
            with tc.tile_pool(name="sp", bufs=1, space="PSUM") as spool, \
                 tc.tile_pool(name="atps", bufs=2, space="PSUM") as atps, \
                 tc.tile_pool(name="avps", bufs=2, space="PSUM") as avps:

                deferred = []

                def make_deferred(g, att, avp):
                    items = []
                    for ci in range(16):
                        def item(ci=ci, att=att, avp=avp):
                            pst = atps.tile([NP, NP], F16, tag="pst", name="pst")
                            nc.tensor.transpose(
                                pst[:], att[:, NP * ci:NP * (ci + 1)], id_sb[:])
                            attT = attTp.tile([NP, NP], F16, tag="attT", name="attT")
                            nc.vector.tensor_copy(out=attT[:], in_=pst[:])
                            attT4 = attT[:].rearrange("v (tt f) -> v tt f", f=4)
                            for b in range(4):
                                nc.tensor.matmul(
                                    avp[32 * b:32 * b + 32, :],
                                    lhsT=attT4[:, :, b],
                                    rhs=Vbuf[:, 1024 * b + HS * ci:1024 * b + HS * (ci + 1)],
                                    start=(ci == 0), stop=(ci == 15),
                                    skip_group_check=True,
                                    tile_position=(0, 32 * b),
                                )
                        items.append(item)

                    def epilogue(g=g, avp=avp):
                        nc.vector.tensor_add(
                            out=outbuf[:, HS * g:HS * (g + 1)], in0=avp[:], in1=bv_sb[:])
                    items.append(epilogue)
                    return items

                relq4 = relq[:].rearrange("p (j i m) -> p j i m", j=NQ, i=2)

                for g in range(G):
                    sp = spool.tile([NP, T], F32, tag="sp", name="sp")
                    # scores (fp16, b-pair contraction-packed)
                    for pair in range(2):
                        for vb in range(NVB):
                            nc.tensor.matmul(
                                sp[:, 512 * vb:512 * (vb + 1)],
                                lhsT=Lsc[:, 256 * g + 128 * pair:256 * g + 128 * (pair + 1)],
                                rhs=kT2[:, T * pair + 512 * vb:T * pair + 512 * (vb + 1)],
                                start=(pair == 0), stop=False,
                                skip_group_check=True,
                            )
                    # rel bias: fp8 DoubleRow quads + drain deferred AV work
                    for jq in range(8):
                        j = 8 * g + jq
                        rpt = rpp.tile([NP, 2 * T], F8, tag="rpt", name="rpt")
                        nc.sync.dma_start(out=rpt[:], in_=rp_d[j, :, :])
                        rpt3 = rpt[:].rearrange("p (i v) -> p i v", i=2)
                        for vb in range(NVB):
                            nc.tensor.matmul(
                                sp[:, 512 * vb:512 * (vb + 1)],
                                lhsT=relq4[:, j],
                                rhs=rpt3[:, :, 512 * vb:512 * (vb + 1)],
                                start=False,
                                stop=(jq == 7 and vb == NVB - 1),
                                perf_mode=DR,
                                skip_group_check=True,
                                tile_position=(0, 0),
                            )
                        if deferred:
                            deferred.pop(0)()
                        if deferred:
                            deferred.pop(0)()
                    while deferred:
                        deferred.pop(0)()
                    # softmax over the free axis
                    mx4 = smx.tile([NP, NVB], F32, tag="mx4", name="mx4")
                    ngm = smx.tile([NP, 1], F32, tag="ngm", name="ngm")
                    z4 = smx.tile([NP, NVB], F32, tag="z4", name="z4")
                    zs = smx.tile([NP, 1], F32, tag="zs", name="zs")
                    rz = smx.tile([NP, 1], F32, tag="rz", name="rz")
                    for vb in range(NVB):
                        nc.vector.reduce_max(
                            out=mx4[:, vb:vb + 1], in_=sp[:, 512 * vb:512 * (vb + 1)], axis=AX)
                    nc.vector.reduce_max(out=ngm[:], in_=mx4[:], axis=AX, negate=True)
                    att = attp.tile([NP, T], F16, tag="att", name="att")
                    for vb in range(NVB):
                        nc.scalar.activation(
                            out=att[:, 512 * vb:512 * (vb + 1)],
                            in_=sp[:, 512 * vb:512 * (vb + 1)],
                            func=EXP, bias=ngm[:, 0:1], scale=1.0)
                    for vb in range(NVB):
                        nc.vector.reduce_sum(
                            out=z4[:, vb:vb + 1], in_=att[:, 512 * vb:512 * (vb + 1)], axis=AX)
                    nc.vector.reduce_sum(out=zs[:], in_=z4[:], axis=AX)
                    nc.vector.reciprocal(rz[:], zs[:])
                    nc.vector.tensor_scalar_mul(att[:], att[:], rz[:, 0:1])
                    if dbg and g == 0:
                        nc.sync.dma_start(out=dbg_att0[:, :], in_=att[:])
                    avp = avps.tile([NP, HS], F32, tag="avp", name="avp")
                    deferred = make_deferred(g, att, avp)
                    if g == G - 1:
                        while deferred:
                            deferred.pop(0)()

                nc.sync.dma_start(out=out_d[:, :], in_=outbuf[:])

    nc.finalize()
    return nc


# ---------------- host side ----------------

def host_prep(x, wq, bq, wk, bk, wv, bv, rel_pos_embd):
    """Build the 8 per-core input dicts."""
    x = np.asarray(x, np.float32)
    rel = np.asarray(rel_pos_embd, np.float32)
    s = np.float32(np.sqrt(np.float32(HS)))

    wq16 = np.ascontiguousarray(np.asarray(wq, np.float32)).astype(np.float16)
    wk16 = np.ascontiguousarray(np.asarray(wk, np.float32) * s).astype(np.float16)
    wv16 = np.ascontiguousarray(np.asarray(wv, np.float32)).astype(np.float16)
    bq2 = np.ascontiguousarray(
        np.tile(np.asarray(bq, np.float32)[None, :], (2, 1)).reshape(NP, 1))
    bk2 = np.ascontiguousarray(
        np.tile(np.asarray(bk, np.float32)[None, :] * s, (2, 1)).reshape(NP, 1))
    bv_rep = np.ascontiguousarray(
        np.tile(np.asarray(bv, np.float32)[None, :], (NP, 1)))
    iden = np.eye(NP, dtype=np.float16)

    xT16 = np.ascontiguousarray(x.transpose(0, 2, 1)).astype(np.float16)  # [B,D,T]

    in_maps = []
    for m in range(NCORES):
        t0 = m * TL
        # rel quad layout: [j, 64*tp+c, i, v] = rel[t0+4j+2i+tp, v, c] * 8
        sl = rel[t0:t0 + TL].reshape(NQ, 2, 2, T, HS)      # (j, i, tp, v, c)
        rp = sl.transpose(0, 2, 4, 1, 3)                    # (j, tp, c, i, v)
        rp8 = (rp.reshape(NQ, NP, 2 * T) * np.float32(8.0)).astype(
            ml_dtypes.float8_e4m3)
        xq16 = np.ascontiguousarray(xT16[:, :, t0:t0 + TL])  # [B, D, TL]
        in_maps.append(dict(
            relp8=np.ascontiguousarray(rp8), xT16=xT16, xq16=xq16,
            wq16=wq16, wk16=wk16, wv16=wv16,
            bq2=bq2, bk8_2=bk2, bv_rep=bv_rep, iden16=iden,
        ))
    return in_maps


def host_unshard(raws):
    """raws: list of 8 out_raw [128, G*HS] -> full [B, T, HS]."""
    out = np.empty((B, T, HS), np.float32)
    for m in range(NCORES):
        t0 = m * TL
        r = np.asarray(raws[m], np.float32).reshape(4, 32, G, HS)
        out[:, t0:t0 + TL, :] = r.transpose(0, 2, 1, 3).reshape(B, TL, HS)
    return out


_NC_CACHE = []


def kernel(**inputs) -> np.ndarray:
    from concourse.bass_utils import run_bass_kernel_spmd

    if not _NC_CACHE:
        _NC_CACHE.append(build_program())
    nc = _NC_CACHE[0]
    in_maps = host_prep(**inputs)
    res = run_bass_kernel_spmd(nc, in_maps, core_ids=list(range(NCORES)))
    raws = [res.results[i]["out_raw"] for i in range(NCORES)]
    return host_unshard(raws)


# revision 6
# speedup vs baseline: 2.9548x; 1.1538x over previous
"""Trainium2 Bass kernel for nn_AttentionHead (B=4, T=2048, D=1024, HS=64).

v5: pipelined stage C (flash-style two-pass softmax over 1024-col halves).

  - Scores + rel accumulate into [128, 1024] PSUM tiles (2 banks, bufs=2):
    half A (v 0..1023) and half B (v 1024..2047) of each 32-query group run
    back-to-back, and the softmax of one half (max -> exp -> sum on
    Vector/Scalar) overlaps the matmuls of the next. The PE never idles
    long enough for the HAM clock gate to re-throttle.
  - Per-half row-max folded into the rel loop (each 512-col bank reduced
    right after its last DoubleRow matmul).
  - Two-pass combine: m = max(mA, mB); z = zA e^(mA-m) + zB e^(mB-m);
    att halves rescaled in place by e^(mX-m)/z before the AV pass.
  - AV swapped: lhsT = V [128v, 64h] stationary, rhs = attT b-slices
    [128, 32] moving -> out [64h, 32b+tt] (half the moving columns).
  - kT2 / Vbuf epilogues on the DVE (exact f32->f16 RNE; the scalar-engine
    copies in v3/v4 degraded k precision on hardware).
  - fp8 e4m3 rel stream (x8 scale) in 2 MiB 4-quad tiles; fp16
    projections/scores; bk dropped (softmax-invariant).

Output raw[h, 128*g + 32*b + tt], t = t0 + 32*g + tt.
"""

import numpy as np
import ml_dtypes

import concourse.bass as bass
import concourse.mybir as mybir
import concourse.tile as tile
from concourse import bacc
from concourse.ap import AP

F32 = mybir.dt.float32
F16 = mybir.dt.float16
F8 = mybir.dt.float8e4
AX = mybir.AxisListType.X
EXP = mybir.ActivationFunctionType.Exp
ADD = mybir.AluOpType.add
MULT = mybir.AluOpType.mult
MIN = mybir.AluOpType.min
DR = mybir.MatmulPerfMode.DoubleRow

B, T, D, HS = 4, 2048, 1024, 64
NCORES = 8
TL = T // NCORES          # 256 query rows per core
G = TL // 32              # 8 groups of 32 t's
NQ = TL // 4              # 64 rel t-quads per core (8 per group)
NP = 128
DC = D // NP
RP_BUFS = 7               # rel prefetch depth (2 MiB 4-quad fp8 tiles)


def build_program(dbg=False):
    nc = bacc.Bacc(None, target_bir_lowering=False, debug=True)

    rp_d = nc.dram_tensor("relp8", [NQ // 4, NP, 8 * T], F8, kind="ExternalInput")
    xT_d = nc.dram_tensor("xT16", [B, D, T], F16, kind="ExternalInput")
    xq_d = nc.dram_tensor("xq16", [B, D, TL], F16, kind="ExternalInput")
    wq_d = nc.dram_tensor("wq16r", [NP, DC * HS], F16, kind="ExternalInput")
    wk_d = nc.dram_tensor("wk16r", [NP, DC * HS], F16, kind="ExternalInput")
    wv_d = nc.dram_tensor("wv16r", [NP, DC * HS], F16, kind="ExternalInput")
    bq_d = nc.dram_tensor("bq2", [NP, 1], F32, kind="ExternalInput")
    bv_d = nc.dram_tensor("bv_col", [64, 1], F32, kind="ExternalInput")
    id_d = nc.dram_tensor("iden16", [NP, NP], F16, kind="ExternalInput")
    out_d = nc.dram_tensor("out_raw", [64, G * NP], F32, kind="ExternalOutput")
    if dbg:
        dbg_kT2 = nc.dram_tensor("dbg_kT2", [NP, 2 * T], F16, kind="ExternalOutput")
        dbg_Lsc = nc.dram_tensor("dbg_Lsc", [NP, G * 256], F16, kind="ExternalOutput")
        dbg_att0 = nc.dram_tensor("dbg_att0", [NP, T], F16, kind="ExternalOutput")

    with tile.TileContext(nc) as tc:
        with tc.tile_pool(name="const", bufs=1) as const, \
             tc.tile_pool(name="persist", bufs=1) as persist, \
             tc.tile_pool(name="xtp", bufs=3) as xtp, \
             tc.tile_pool(name="xqp", bufs=2) as xqp, \
             tc.tile_pool(name="rpp", bufs=RP_BUFS) as rpp, \
             tc.tile_pool(name="attp", bufs=2) as attp, \
             tc.tile_pool(name="attTp", bufs=4) as attTp, \
             tc.tile_pool(name="smx", bufs=3) as smx:

            # ---- constants ----
            wq_sb = const.tile([NP, DC * HS], F16, tag="wq", name="wq")
            wk_sb = const.tile([NP, DC * HS], F16, tag="wk", name="wk")
            wv_sb = const.tile([NP, DC * HS], F16, tag="wv", name="wv")
            nc.sync.dma_start(out=wq_sb[:], in_=wq_d[:, :])
            nc.sync.dma_start(out=wk_sb[:], in_=wk_d[:, :])
            nc.sync.dma_start(out=wv_sb[:], in_=wv_d[:, :])
            bq_sb = const.tile([NP, 1], F32, tag="bq", name="bq")
            bv_sb = const.tile([64, 1], F32, tag="bv", name="bv")
            id_sb = const.tile([NP, NP], F16, tag="iden", name="iden")
            nc.sync.dma_start(out=bq_sb[:], in_=bq_d[:, :])
            nc.sync.dma_start(out=bv_sb[:], in_=bv_d[:, :])
            nc.sync.dma_start(out=id_sb[:], in_=id_d[:, :])

            # ---- persistent activations ----
            kT2 = persist.tile([NP, 2 * T], F16, tag="kT2", name="kT2")
            Vbuf = persist.tile([NP, B * 16 * HS], F16, tag="Vbuf", name="Vbuf")
            Lsc = persist.tile([NP, G * 256], F16, tag="Lsc", name="Lsc")
            relq = persist.tile([NP, NQ * 256], F8, tag="relq", name="relq")
            outbuf = persist.tile([64, G * NP], F32, tag="outbuf", name="outbuf")
            zero1 = persist.tile([NP, 1], F32, tag="zero1", name="zero1")
            nc.gpsimd.memset(zero1[:], 0.0)
            nc.gpsimd.memset(Lsc[:], 0.0)
            nc.vector.memset(relq[:], 0.0)

            LSC_PITCH = G * 256
            RELQ_PITCH = NQ * 256

            # =========== stage A: q projection + scatters ===========
            with tc.tile_pool(name="qps", bufs=2, space="PSUM") as qps:
                for b in range(B):
                    h = b % 2
                    psq = qps.tile([NP, TL], F32, tag="psq", name="psq")
                    xq = xqp.tile([NP, DC * TL], F16, tag="xq", name="xq")
                    nc.sync.dma_start(
                        out=xq[:].rearrange("p (dc t) -> p dc t", dc=DC),
                        in_=xq_d[b].rearrange("(dc p) t -> p dc t", p=NP))
                    for dc in range(DC):
                        for half in range(2):
                            nc.tensor.matmul(
                                psq[64 * half:64 * half + 64, :],
                                lhsT=wq_sb[:, HS * dc:HS * (dc + 1)],
                                rhs=xq[:, TL * dc:TL * (dc + 1)],
                                start=(dc == 0), stop=(dc == DC - 1),
                                skip_group_check=True,
                                tile_position=(0, 64 * half),
                            )
                    for hh in range(2):
                        rows = slice(64 * hh, 64 * hh + 64)
                        if hh == h:
                            dst = AP(Lsc[:].tensor,
                                     64 * hh * LSC_PITCH + 128 * (b // 2) + b,
                                     [[LSC_PITCH, 64], [256, G], [4, 32]])
                            src = AP(psq[:].tensor, 64 * hh * TL,
                                     [[TL, 64], [32, G], [1, 32]])
                            nc.vector.tensor_scalar_add(
                                dst, src, bq_sb[rows, 0:1])
                        tp = hh
                        dstq = AP(relq[:].tensor,
                                  64 * tp * RELQ_PITCH + 4 * tp + b,
                                  [[RELQ_PITCH, 64], [2048, G],
                                   [272, 8], [136, 2]])
                        srcq = AP(psq[:].tensor, 64 * tp * TL + tp,
                                  [[TL, 64], [32, G], [4, 8], [2, 2]])
                        nc.vector.tensor_scalar(
                            dstq, srcq, bq_sb[rows, 0:1], 0.125,
                            op0=ADD, op1=MULT)

            # =========== stage B: k / v projections ===========
            with tc.tile_pool(name="kps", bufs=1, space="PSUM") as kps, \
                 tc.tile_pool(name="vps", bufs=2, space="PSUM") as vps:
                for bp in range(2):
                    psk = kps.tile([NP, T], F32, tag="psk", name="psk")
                    for b in (2 * bp, 2 * bp + 1):
                        h = b % 2
                        psv = vps.tile([NP, 16 * HS], F32, tag="psv", name="psv")
                        for dj in range(DC // 2):
                            xt2 = xtp.tile([NP, 2 * T], F16, tag="xt", name="xt")
                            nc.sync.dma_start(
                                out=xt2[:].rearrange("p (dp t) -> p dp t", dp=2),
                                in_=xT_d[b, 2 * dj * NP:2 * (dj + 1) * NP, :]
                                    .rearrange("(dp p) t -> p dp t", dp=2))
                            for dp in range(2):
                                dc = 2 * dj + dp
                                xt = xt2[:, T * dp:T * (dp + 1)]
                                for vb in range(4):
                                    nc.tensor.matmul(
                                        psk[64 * h:64 * h + 64, 512 * vb:512 * (vb + 1)],
                                        lhsT=wk_sb[:, HS * dc:HS * (dc + 1)],
                                        rhs=xt[:, 512 * vb:512 * (vb + 1)],
                                        start=(dc == 0), stop=(dc == DC - 1),
                                        skip_group_check=True,
                                        tile_position=(0, 64 * h),
                                    )
                                for ci in range(16):
                                    nc.tensor.matmul(
                                        psv[:, HS * ci:HS * (ci + 1)],
                                        lhsT=xt[:, NP * ci:NP * (ci + 1)],
                                        rhs=wv_sb[:, HS * dc:HS * (dc + 1)],
                                        start=(dc == 0 and ci % 8 == 0),
                                        stop=(dc == DC - 1),
                                        skip_group_check=True,
                                    )
                        nc.vector.tensor_copy(
                            out=Vbuf[:, 1024 * b:1024 * (b + 1)], in_=psv[:])
                        nc.vector.tensor_copy(
                            out=kT2[64 * h:64 * h + 64, T * bp:T * (bp + 1)],
                            in_=psk[64 * h:64 * h + 64, :])

            # =========== stage C: pipelined scores+rel+softmax+AV ===========
            with tc.tile_pool(name="sp", bufs=2, space="PSUM") as spool, \
                 tc.tile_pool(name="atps", bufs=2, space="PSUM") as atps, \
                 tc.tile_pool(name="avps", bufs=2, space="PSUM") as avps:

                deferred = []

                def drain(k):
                    for _ in range(k):
                        if deferred:
                            deferred.pop(0)()

                def make_deferred(g, att, avp):
                    items = []
                    for ci in range(16):
                        def item(ci=ci, att=att, avp=avp):
                            pst = atps.tile([NP, NP], F16, tag="pst", name="pst")
                            nc.tensor.transpose(
                                pst[:], att[:, NP * ci:NP * (ci + 1)], id_sb[:])
                            attT = attTp.tile([NP, NP], F16, tag="attT", name="attT")
                            if ci % 2 == 0:
                                nc.vector.tensor_copy(out=attT[:], in_=pst[:])
                            else:
                                nc.scalar.activation(
                                    out=attT[:], in_=pst[:],
                                    func=mybir.ActivationFunctionType.Copy)
                            attT4 = attT[:].rearrange("v (tt f) -> v tt f", f=4)
                            for b in range(4):
                                # one start=True per PSUM bank: it poisons the
                                # whole bank pending-zero, so the other b
                                # blocks' first writes overwrite correctly
                                nc.tensor.matmul(
                                    avp[:, 32 * b:32 * (b + 1)],
                                    lhsT=Vbuf[:, 1024 * b + HS * ci:1024 * b + HS * (ci + 1)],
                                    rhs=attT4[:, :, b],
                                    start=(ci == 0 and b == 0), stop=(ci == 15),
                                    skip_group_check=True,
                                    tile_position=(0, 0),
                                )
                        items.append(item)

                    def epilogue(g=g, avp=avp):
                        nc.vector.tensor_scalar_add(
                            outbuf[:, NP * g:NP * (g + 1)], avp[:],
                            bv_sb[:, 0:1])
                    items.append(epilogue)
                    return items

                relq4 = relq[:].rearrange("p (j i m) -> p j i m", j=NQ, i=2)

                for g in range(G):
                    rpts = []
                    for jt in range(2):
                        rpt = rpp.tile([NP, 8 * T], F8, tag="rpt", name="rpt")
                        nc.sync.dma_start(out=rpt[:], in_=rp_d[2 * g + jt, :, :])
                        rpts.append(rpt[:].rearrange(
                            "p (q i v) -> p q i v", q=4, i=2))
                    att = attp.tile([NP, T], F16, tag="att", name="att")
                    ngm2 = smx.tile([NP, 2], F32, tag="ngm2", name="ngm2")
                    zs2 = smx.tile([NP, 2], F32, tag="zs2", name="zs2")
                    mx2 = smx.tile([NP, 2], F32, tag="mx2", name="mx2")

                    for X in range(2):          # half A: v 0..1023, B: rest
                        sp = spool.tile([NP, T // 2], F32, tag="sp", name="sp")
                        for pair in range(2):
                            for vbl in range(2):
                                vb = 2 * X + vbl
                                nc.tensor.matmul(
                                    sp[:, 512 * vbl:512 * (vbl + 1)],
                                    lhsT=Lsc[:, 256 * g + 128 * pair:256 * g + 128 * (pair + 1)],
                                    rhs=kT2[:, T * pair + 512 * vb:T * pair + 512 * (vb + 1)],
                                    start=(pair == 0), stop=False,
                                    skip_group_check=True,
                                )
                        for jt in range(2):
                            for jq4 in range(4):
                                jq = 4 * jt + jq4
                                j = 8 * g + jq
                                for vbl in range(2):
                                    vb = 2 * X + vbl
                                    nc.tensor.matmul(
                                        sp[:, 512 * vbl:512 * (vbl + 1)],
                                        lhsT=relq4[:, j],
                                        rhs=rpts[jt][:, jq4, :, 512 * vb:512 * (vb + 1)],
                                        start=False,
                                        stop=(jq == 7 and True),
                                        perf_mode=DR,
                                        skip_group_check=True,
                                        tile_position=(0, 0),
                                    )
                                    if jq == 7:
                                        nc.vector.reduce_max(
                                            out=mx2[:, vbl:vbl + 1],
                                            in_=sp[:, 512 * vbl:512 * (vbl + 1)],
                                            axis=AX)
                                drain(1)
                        nc.vector.reduce_max(
                            out=ngm2[:, X:X + 1], in_=mx2[:], axis=AX,
                            negate=True)
                        nc.scalar.activation(
                            out=att[:, 1024 * X:1024 * (X + 1)], in_=sp[:],
                            func=EXP, bias=ngm2[:, X:X + 1], scale=1.0,
                            accum_out=zs2[:, X:X + 1])
                    # combine halves: ngmM = min(ngmA, ngmB) = -(max m);
                    # eX = exp(-(ngmX - ngmM)); z = sum zX eX; scl X = eX/z
                    ngmM = smx.tile([NP, 1], F32, tag="ngmM", name="ngmM")
                    e2 = smx.tile([NP, 2], F32, tag="e2", name="e2")
                    z2 = smx.tile([NP, 2], F32, tag="z2", name="z2")
                    zsum = smx.tile([NP, 1], F32, tag="zsum", name="zsum")
                    rz = smx.tile([NP, 1], F32, tag="rz", name="rz")
                    scl2 = smx.tile([NP, 2], F32, tag="scl2", name="scl2")
                    nc.vector.tensor_reduce(
                        out=ngmM[:], in_=ngm2[:], axis=AX, op=MIN)
                    nc.vector.tensor_scalar_sub(e2[:], ngm2[:], ngmM[:, 0:1])
                    nc.scalar.activation(out=e2[:], in_=e2[:], func=EXP,
                                         bias=zero1[:, 0:1], scale=-1.0)
                    nc.vector.tensor_mul(out=z2[:], in0=zs2[:], in1=e2[:])
                    nc.vector.reduce_sum(out=zsum[:], in_=z2[:], axis=AX)
                    nc.vector.reciprocal(rz[:], zsum[:])
                    nc.vector.tensor_scalar_mul(scl2[:], e2[:], rz[:, 0:1])
                    for X in range(2):
                        nc.vector.tensor_scalar_mul(
                            att[:, 1024 * X:1024 * (X + 1)],
                            att[:, 1024 * X:1024 * (X + 1)],
                            scl2[:, X:X + 1])
                    if dbg and g == 0:
                        nc.sync.dma_start(out=dbg_att0[:, :], in_=att[:])
                    avp = avps.tile([64, NP], F32, tag="avp", name="avp")
                    deferred.extend(make_deferred(g, att, avp))
                    if g == G - 1:
                        drain(len(deferred))

                nc.sync.dma_start(out=out_d[:, :], in_=outbuf[:])
                if dbg:
                    nc.sync.dma_start(out=dbg_kT2[:, :], in_=kT2[:])
                    nc.sync.dma_start(out=dbg_Lsc[:, :], in_=Lsc[:])

    nc.finalize()
    return nc


# ---------------- host side ----------------

def host_prep(x, wq, bq, wk, bk, wv, bv, rel_pos_embd):
    """Build the 8 per-core input dicts."""
    x = np.asarray(x, np.float32)
    rel = np.asarray(rel_pos_embd, np.float32)
    s = np.float32(np.sqrt(np.float32(HS)))

    def warr(w, scale=None):
        w = np.asarray(w, np.float32)
        if scale is not None:
            w = w * scale
        return np.ascontiguousarray(
            w.reshape(DC, NP, HS).transpose(1, 0, 2).reshape(NP, DC * HS)
        ).astype(np.float16)

    wq16 = warr(wq)
    wk16 = warr(wk, s)     # sqrt(HS) folded into k; bk dropped (softmax-inv)
    wv16 = warr(wv)
    bq2 = np.ascontiguousarray(
        np.tile(np.asarray(bq, np.float32)[None, :], (2, 1)).reshape(NP, 1))
    bv_col = np.ascontiguousarray(np.asarray(bv, np.float32).reshape(64, 1))
    iden = np.eye(NP, dtype=np.float16)

    xT16 = np.ascontiguousarray(x.transpose(0, 2, 1)).astype(np.float16)

    in_maps = []
    for m in range(NCORES):
        t0 = m * TL
        sl = rel[t0:t0 + TL].reshape(NQ, 2, 2, T, HS)      # (j, i, tp, v, c)
        rp = sl.transpose(0, 2, 4, 1, 3)                    # (j, tp, c, i, v)
        rp8 = (rp.reshape(NQ // 4, 4, NP, 2 * T).transpose(0, 2, 1, 3)
               .reshape(NQ // 4, NP, 8 * T) * np.float32(8.0)).astype(
            ml_dtypes.float8_e4m3)
        xq16 = np.ascontiguousarray(xT16[:, :, t0:t0 + TL])
        in_maps.append(dict(
            relp8=np.ascontiguousarray(rp8), xT16=xT16, xq16=xq16,
            wq16r=wq16, wk16r=wk16, wv16r=wv16,
            bq2=bq2, bv_col=bv_col, iden16=iden,
        ))
    return in_maps


def host_unshard(raws):
    """raws: list of 8 out_raw [64, G*128] -> full [B, T, HS]."""
    out = np.empty((B, T, HS), np.float32)
    for m in range(NCORES):
        t0 = m * TL
        r = np.asarray(raws[m], np.float32).reshape(HS, G, 4, 32)
        out[:, t0:t0 + TL, :] = r.transpose(2, 1, 3, 0).reshape(B, TL, HS)
    return out


_NC_CACHE = []


def kernel(**inputs) -> np.ndarray:
    from concourse.bass_utils import run_bass_kernel_spmd

    if not _NC_CACHE:
        _NC_CACHE.append(build_program())
    nc = _NC_CACHE[0]
    in_maps = host_prep(**inputs)
    res = run_bass_kernel_spmd(nc, in_maps, core_ids=list(range(NCORES)))
    raws = [res.results[i]["out_raw"] for i in range(NCORES)]
    return host_unshard(raws)
